# revision 1
# baseline (speedup 1.0000x reference)
"""CRF loss kernel for Trainium2 (8 NeuronCores).

Strategy
--------
The CRF forward scan  Z_{t+1} = logsumexp_i(Z_t[:,i] + Tr[i,j]) + logit_{t+1}
is rewritten in exp-domain as a *normalized* plain matmul recurrence:

    a_{t+1} = (a_t @ exp(Tr)) * exp(logit_{t+1})        (kept unnormalized
    U_t     = sum_j a_t[b, j]                            for up to R steps,
    lse_j Z_t = ln U_t + D_t                             renormalize by U and
                                                         fold ln U into D)

which runs on the TensorEngine: per step, 64 matmuls of
[K=128]x[M=128]x[N=32] with the exp(transition) tiles as stationary
operands (fp8, fast-weight-load) and the transposed state a^T [i, b] as
the moving operand (bf16).  log Z[b] is harvested at t == len[b]-1 via a
precomputed one-hot selector.  The projection logits = enc @ W.T + b is a
dense matmul producing the *transposed* layout [V, T, B] the scan consumes.
The gold-path score uses indirect-DMA element gathers from logits / transition.

The scan is inherently sequential over T and too small in B to shard
without per-step cross-core communication (collective latency >> step
time), so each core runs the identical full problem (data-parallel
replication costs nothing in wall time) and the host takes core 0's
scalar output.
"""
import sys
import os

sys.path.insert(0, "/opt/trn_rl_repo")

import numpy as np
import ml_dtypes

import concourse.bass as bass
import concourse.bacc as bacc
import concourse.tile as tile
from concourse import mybir
from concourse.bass_utils import run_bass_kernel_spmd

T, B, H, V = 256, 32, 512, 1024
P = 128
JC = V // P          # 8 vocab chunks
KH = H // P          # 4 hidden chunks
NTB = (T * B) // 512  # 16 tb-chunks of 512 for the projection
TB = T * B
R = 4                # renormalize the scan state every R steps
N_CORES = 8

F32 = mybir.dt.float32
BF16 = mybir.dt.bfloat16
FP8 = mybir.dt.float8e4

EHAT_DT = FP8        # dtype of exp(transition) stationary tiles
SCAN_STEPS = T       # full scan


def _build_program(steps=SCAN_STEPS, ehat_dt=EHAT_DT, skip_gather=False,
                   skip_renorm=False):
    nc = bacc.Bacc("TRN2", target_bir_lowering=False, debug=False,
                   enable_asserts=False, num_devices=N_CORES)

    encT_d = nc.dram_tensor("encT", [H, TB], BF16, kind="ExternalInput")
    wt_d = nc.dram_tensor("Wt", [H, V], F32, kind="ExternalInput")
    bcol_d = nc.dram_tensor("bcol", [P, JC], F32, kind="ExternalInput")
    trans_d = nc.dram_tensor("trans", [V, V], F32, kind="ExternalInput")
    selT_d = nc.dram_tensor("selT", [1, TB], F32, kind="ExternalInput")
    eoff_d = nc.dram_tensor("eoff", [P, 64], mybir.dt.int32, kind="ExternalInput")
    esel_d = nc.dram_tensor("esel", [P, 64, B], F32, kind="ExternalInput")
    toff_d = nc.dram_tensor("toff", [P, 64], mybir.dt.int32, kind="ExternalInput")
    tsel_d = nc.dram_tensor("tsel", [P, 64, B], F32, kind="ExternalInput")
    foldE_d = nc.dram_tensor("foldE", [P, B], F32, kind="ExternalInput")
    loss_d = nc.dram_tensor("loss", [1, 1], F32, kind="ExternalOutput")

    with tile.TileContext(nc) as tc:
        with tc.tile_pool(name="const", bufs=1) as cp, \
             tc.tile_pool(name="dram", bufs=1, space="DRAM") as dp:
            logits_t = dp.tile([V, T, B], BF16, tag="logits")

            # ---------------- phase A: constants -------------------------
            ehat = cp.tile([P, JC, V], ehat_dt, tag="ehat")
            wt_sb = cp.tile([P, KH, V], BF16, tag="wt")
            ones_c = cp.tile([P, 1], BF16, tag="ones_c")
            ones_r = cp.tile([1, P], F32, tag="ones_r")
            selT_sb = cp.tile([1, TB], F32, tag="selT")
            bcol_sb = cp.tile([P, JC], F32, tag="bcol")
            logZ = cp.tile([1, B], F32, tag="logZ")
            Dacc = cp.tile([1, B], F32, tag="Dacc")
            eoff_sb = cp.tile([P, 64], mybir.dt.int32, tag="eoff")
            esel_sb = cp.tile([P, 64, B], F32, tag="esel")
            toff_sb = cp.tile([P, 64], mybir.dt.int32, tag="toff")
            tsel_sb = cp.tile([P, 64, B], F32, tag="tsel")
            foldE_sb = cp.tile([P, B], F32, tag="foldE")

            nc.vector.memset(ones_c[:], 1.0)
            nc.vector.memset(ones_r[:], 1.0)
            nc.vector.memset(logZ[:], 0.0)
            nc.vector.memset(Dacc[:], 0.0)
            nc.sync.dma_start(selT_sb[:], selT_d.ap()[:])
            nc.sync.dma_start(bcol_sb[:], bcol_d.ap()[:])
            nc.sync.dma_start(eoff_sb[:], eoff_d.ap()[:])
            nc.sync.dma_start(esel_sb[:], esel_d.ap()[:])
            nc.sync.dma_start(toff_sb[:], toff_d.ap()[:])
            nc.sync.dma_start(tsel_sb[:], tsel_d.ap()[:])
            nc.sync.dma_start(foldE_sb[:], foldE_d.ap()[:])

            with tc.tile_pool(name="stage", bufs=2) as stp:
                for k in range(JC):
                    st = stp.tile([P, V], F32, tag="stg")
                    nc.sync.dma_start(st[:], trans_d.ap()[k * P:(k + 1) * P, :])
                    nc.scalar.activation(ehat[:, k, :], st[:],
                                         mybir.ActivationFunctionType.Exp)
                for k in range(KH):
                    st = stp.tile([P, V], F32, tag="stg")
                    nc.sync.dma_start(st[:], wt_d.ap()[k * P:(k + 1) * P, :])
                    nc.vector.tensor_copy(wt_sb[:, k, :], st[:])

            # ---------------- phase B: projection ------------------------
            # logits[v, t, b] = sum_h W[v, h] enc[t, b, h] + b[v]
            with tc.tile_pool(name="proj_ps", bufs=3, space="PSUM") as pps, \
                 tc.tile_pool(name="enc_p", bufs=2) as encp, \
                 tc.tile_pool(name="lg_p", bufs=3) as lgp:
                for n in range(NTB):
                    enc_tiles = []
                    for k in range(KH):
                        et = encp.tile([P, 512], BF16, tag=f"enc{k}")
                        nc.sync.dma_start(
                            et[:], encT_d.ap()[k * P:(k + 1) * P,
                                               n * 512:(n + 1) * 512])
                        enc_tiles.append(et)
                    for m in range(JC):
                        ps = pps.tile([P, 512], F32, tag="pps")
                        for k in range(KH):
                            nc.tensor.matmul(ps[:], lhsT=wt_sb[:, k, m * P:(m + 1) * P],
                                             rhs=enc_tiles[k][:],
                                             start=(k == 0), stop=(k == KH - 1))
                        lg = lgp.tile([P, 16, B], BF16, tag="lg")
                        nc.vector.tensor_scalar_add(
                            lg[:], ps[:].rearrange("p (t b) -> p t b", t=16),
                            bcol_sb[:, m:m + 1])
                        nc.sync.dma_start(
                            logits_t[m * P:(m + 1) * P, n * 16:(n + 1) * 16, :],
                            lg[:])

            # ---------------- gold-path score (overlaps the scan) --------
            with tc.tile_pool(name="gth", bufs=3) as gp, \
                 tc.tile_pool(name="fold_ps", bufs=1, space="PSUM") as fps:
                t_fold = fps.tile([1, B], F32, tag="tf")
                e_fold = fps.tile([1, B], F32, tag="ef")
                if skip_gather:
                    nc.vector.memset(t_fold[:], 0.0)
                    nc.vector.memset(e_fold[:], 0.0)
                else:
                    # row gathers of 32-wide rows, one [P,1] offset col per
                    # call (HW supports exactly one offset per partition);
                    # per-call column-select masks (mask pre-folded) reduce
                    # each gathered [P,32] row to acc[:, c].
                    def gather_fold(src_rows32, off_sb, sel_sb, gdt, fold_out,
                                    nm):
                        acc = gp.tile([P, 64], F32, tag=f"{nm}acc")
                        nc.vector.memset(acc[:], 0.0)
                        for c in range(64):
                            g = gp.tile([P, B], gdt, tag=f"{nm}g")
                            nc.gpsimd.indirect_dma_start(
                                out=g[:], out_offset=None, in_=src_rows32,
                                in_offset=bass.IndirectOffsetOnAxis(
                                    ap=off_sb[:, c:c + 1], axis=0))
                            scr = gp.tile([P, B], F32, tag=f"{nm}scr")
                            nc.vector.tensor_tensor(
                                out=scr[:], in0=g[:], in1=sel_sb[:, c, :],
                                op=mybir.AluOpType.mult)
                            nc.vector.tensor_reduce(
                                out=acc[:, c:c + 1], in_=scr[:],
                                axis=mybir.AxisListType.X,
                                op=mybir.AluOpType.add)
                        ssum = gp.tile([P, 1], F32, tag=f"{nm}sum")
                        nc.vector.tensor_reduce(
                            out=ssum[:], in_=acc[:],
                            axis=mybir.AxisListType.X, op=mybir.AluOpType.add)
                        nc.tensor.matmul(fold_out[:], lhsT=ssum[:],
                                         rhs=foldE_sb[:], start=True, stop=True)

                    tv = trans_d.ap()
                    trows = bass.AP(tv.tensor, tv.offset,
                                    [[B, V * V // B], [1, B]])
                    gather_fold(trows, toff_sb, tsel_sb, F32, t_fold, "t")

                    lv = logits_t[:]
                    lrows = bass.AP(lv.tensor, lv.offset,
                                    [[B, V * T], [1, B]])
                    gather_fold(lrows, eoff_sb, esel_sb, BF16, e_fold, "e")

                # ---------------- phase C: the scan ----------------------
                lview = logits_t[:].rearrange("(jc p) t b -> p jc t b", p=P)
                with tc.tile_pool(name="scan_sb", bufs=3) as ssb, \
                     tc.tile_pool(name="lt_p", bufs=2) as ltp, \
                     tc.tile_pool(name="s_ps", bufs=2, space="PSUM") as sps, \
                     tc.tile_pool(name="u_ps", bufs=2, space="PSUM") as ups, \
                     tc.tile_pool(name="bc_ps", bufs=2, space="PSUM") as bps, \
                     tc.tile_pool(name="sm", bufs=4) as smp:

                    def load_lt(tc0):
                        lt = ltp.tile([P, JC, 8, B], BF16, tag="lt")
                        nc.sync.dma_start(lt[:], lview[:, :, tc0:tc0 + 8, :])
                        return lt

                    def harvest(a_cur, t, renorm):
                        """U, lse bookkeeping + optional renormalize."""
                        u8 = ups.tile([1, JC, B], F32, tag="u8")
                        nc.tensor.matmul(u8[:], lhsT=ones_c[:], rhs=a_cur[:],
                                         start=True, stop=True)
                        U = smp.tile([1, B], F32, tag="U")
                        nc.vector.tensor_reduce(
                            out=U[:], in_=u8[:].rearrange("p k b -> p b k"),
                            axis=mybir.AxisListType.X, op=mybir.AluOpType.add)
                        lnU = smp.tile([1, B], F32, tag="lnU")
                        nc.scalar.activation(lnU[:], U[:],
                                             mybir.ActivationFunctionType.Ln)
                        lam = smp.tile([1, B], F32, tag="lam")
                        nc.vector.tensor_add(lam[:], lnU[:], Dacc[:])
                        tmp = smp.tile([1, B], F32, tag="tmp")
                        nc.vector.tensor_mul(tmp[:], lam[:],
                                             selT_sb[:, t * B:(t + 1) * B])
                        nc.vector.tensor_add(logZ[:], logZ[:], tmp[:])
                        if not renorm or skip_renorm:
                            return a_cur
                        invU = smp.tile([1, B], F32, tag="invU")
                        nc.vector.reciprocal(invU[:], U[:])
                        bc = bps.tile([P, B], F32, tag="bc")
                        nc.tensor.matmul(bc[:], lhsT=ones_r[:], rhs=invU[:],
                                         start=True, stop=True)
                        a_n = ssb.tile([P, JC, B], BF16, tag="a")
                        bca = bc[:]
                        bc_bcast = bass.AP(
                            bca.tensor, bca.offset,
                            [list(bca.ap[0]), [0, JC], list(bca.ap[1])])
                        nc.vector.tensor_tensor(
                            out=a_n[:], in0=a_cur[:], in1=bc_bcast,
                            op=mybir.AluOpType.mult)
                        nc.vector.tensor_add(Dacc[:], Dacc[:], lnU[:])
                        return a_n

                    # t = 0
                    lt = load_lt(0)
                    a_prev = ssb.tile([P, JC, B], BF16, tag="a")
                    nc.scalar.activation(a_prev[:], lt[:, :, 0, :],
                                         mybir.ActivationFunctionType.Exp)
                    a_prev = harvest(a_prev, 0, renorm=False)

                    for t in range(1, steps):
                        if t % 8 == 0:
                            lt = load_lt(t)
                        g = smp.tile([P, JC, B], BF16, tag="g")
                        nc.scalar.activation(g[:], lt[:, :, t % 8, :],
                                             mybir.ActivationFunctionType.Exp)
                        s_ps = sps.tile([P, JC, B], F32, tag="s")
                        a_cur = ssb.tile([P, JC, B], BF16, tag="a")
                        for m in range(JC):
                            for k in range(JC):
                                nc.tensor.matmul(
                                    s_ps[:, m, :],
                                    lhsT=ehat[:, k, m * P:(m + 1) * P],
                                    rhs=a_prev[:, k, :],
                                    start=(k == 0), stop=(k == JC - 1))
                            nc.vector.tensor_tensor(
                                out=a_cur[:, m, :], in0=s_ps[:, m, :],
                                in1=g[:, m, :], op=mybir.AluOpType.mult)
                        a_prev = harvest(a_cur, t,
                                         renorm=(t % R == R - 1 and
                                                 t != steps - 1))

                    # ---------------- finalize ---------------------------
                    d1 = smp.tile([1, B], F32, tag="d1")
                    nc.vector.tensor_tensor(out=d1[:], in0=logZ[:],
                                            in1=e_fold[:],
                                            op=mybir.AluOpType.subtract)
                    d2 = smp.tile([1, B], F32, tag="d2")
                    nc.vector.tensor_tensor(out=d2[:], in0=d1[:],
                                            in1=t_fold[:],
                                            op=mybir.AluOpType.subtract)
                    tot = smp.tile([1, 1], F32, tag="tot")
                    nc.vector.tensor_reduce(out=tot[:], in_=d2[:],
                                            axis=mybir.AxisListType.X,
                                            op=mybir.AluOpType.add)
                    res = smp.tile([1, 1], F32, tag="res")
                    nc.vector.tensor_scalar_mul(res[:], tot[:], 1.0 / B)
                    nc.sync.dma_start(loss_d.ap()[:], res[:])

    nc.compile()
    return nc


_CACHE = {}


def _get_program():
    if "nc" not in _CACHE:
        _CACHE["nc"] = _build_program()
    return _CACHE["nc"]


def _stage_inputs(enc_outs, W, b, transition, targets, lengths):
    enc_outs = np.asarray(enc_outs, dtype=np.float32)
    W = np.asarray(W, dtype=np.float32)
    b = np.asarray(b, dtype=np.float32)
    transition = np.asarray(transition, dtype=np.float32)
    targets = np.asarray(targets, dtype=np.int32)
    lengths = np.asarray(lengths, dtype=np.int32)

    encT = np.ascontiguousarray(
        enc_outs.transpose(2, 0, 1).reshape(H, TB)).astype(ml_dtypes.bfloat16)
    Wt = np.ascontiguousarray(W.T)
    bcol = np.ascontiguousarray(b.reshape(JC, P).T)

    tt = np.arange(T)[:, None]                      # [T, 1]
    mask = (tt < lengths[None, :]).astype(np.float32)        # [T, B]
    sel = (tt == (lengths[None, :] - 1)).astype(np.float32)  # [T, B]
    selT = np.ascontiguousarray(sel.reshape(1, TB))

    # gather layouts: pair q = t*B + b -> (p, c) = (q % 128, q // 128).
    # Each gather call c fetches, for every partition p, a 32-wide row of
    # the flat source; sel[p, c, :] one-hot-selects the wanted column with
    # the sequence mask folded in.
    tgrid = np.repeat(np.arange(T), B)              # [TB]
    bgrid = np.tile(np.arange(B), T)                # [TB]
    tgt_flat = targets.reshape(TB).astype(np.int64)  # targets[t, b] at q

    onehot = np.eye(B, dtype=np.float32)

    # emit: logits[V, T, B] flat row = v*T + t, col = b
    eoff = (tgt_flat * T + tgrid).astype(np.int32)
    eoff = eoff.reshape(64, P).T.copy()             # [P, 64]
    esel = (mask.reshape(TB)[:, None] * onehot[bgrid]).astype(np.float32)
    esel = np.ascontiguousarray(
        esel.reshape(64, P, B).transpose(1, 0, 2))  # [P, 64, B]

    # trans: for t < T-1: trans[targets[t,b], targets[t+1,b]]
    # flat row = (tgt_t*V + tgt_t1)//32, col = tgt_t1 % 32; padded with 0
    toff = np.zeros(TB, np.int64)
    tself = np.zeros((TB, B), np.float32)
    q = tgrid < T - 1
    t_idx = tgrid[q]
    b_idx = bgrid[q]
    tgt0 = targets[t_idx, b_idx].astype(np.int64)
    tgt1 = targets[t_idx + 1, b_idx].astype(np.int64)
    toff[q] = tgt0 * (V // B) + tgt1 // B
    tself[q] = mask[t_idx + 1, b_idx][:, None] * onehot[tgt1 % B]
    toff = toff.reshape(64, P).T.astype(np.int32).copy()
    tsel = np.ascontiguousarray(tself.reshape(64, P, B).transpose(1, 0, 2))

    foldE = (np.arange(P)[:, None] % B == np.arange(B)[None, :]).astype(np.float32)

    return {
        "encT": encT, "Wt": Wt, "bcol": bcol, "trans": transition,
        "selT": selT, "eoff": eoff, "esel": esel, "toff": toff,
        "tsel": tsel, "foldE": foldE,
    }


def kernel(enc_outs, W, b, transition, targets, lengths):
    nc = _get_program()
    in_map = _stage_inputs(enc_outs, W, b, transition, targets, lengths)
    in_maps = [in_map for _ in range(N_CORES)]
    res = run_bass_kernel_spmd(nc, in_maps, core_ids=list(range(N_CORES)))
    loss = res.results[0]["loss"]
    return np.float32(loss.reshape(())[()])

